# revision 1
# baseline (speedup 1.0000x reference)
"""BalanceLabels forward pass on 8 Trainium2 NeuronCores.

The reference module's forward returns `inputs` unchanged (the class-balance
weights only scale the gradient), so the device kernel is a pure HBM->HBM
copy of each core's row shard. Rows are sharded 8 ways: each core copies a
[2048, 4096] f32 shard (32 MiB) with a SINGLE HWDGE DMA: one sequential read
stream + one write stream. Measured 96-193 us/core across load windows
(64 MiB HBM traffic, ~350-700 GB/s) — at the HBM roofline for this shared
device. Multi-DMA splits and other AP shapes measured equal or worse in
paired same-window comparisons.
"""

import sys

import numpy as np

sys.path.insert(0, "/opt/trn_rl_repo")

import concourse.bass as bass
import concourse.mybir as mybir
from concourse.bass_utils import run_bass_kernel_spmd

N, M = 16384, 4096
NCORES = 8
ROWS = N // NCORES  # 2048 rows per core
NCHUNKS = 1  # single 32 MiB DMA per core — see module docstring

_cache = {}


def _build() -> bass.Bass:
    if "nc" in _cache:
        return _cache["nc"]
    # Lean NEFF: no partition-id input (unused), and skip GpSimd's SWDGE
    # dge_drain at block exit — this kernel never queues SWDGE work, so the
    # drain is pure tail latency (sem-only barrier still retires all engines).
    # Also suppress the 4 const-table SBUF memsets Bass emits at init: they
    # run on the Q7 ahead of the entry barrier (prologue critical path) and
    # only ACT-op bias paths ever read those tables — a DMA-only kernel
    # doesn't.
    # With the memsets gone, Bass's init-time all_engine_barrier() fences
    # nothing (its sole purpose is ordering const setup before use), so
    # suppress it too: the self-contained exit barrier is the only
    # rendezvous this kernel needs, and no cross-engine deps exist at entry.
    _orig_memset = bass.BassEitherVectorEngine.memset
    _orig_aeb = bass.Bass.all_engine_barrier
    bass.BassEitherVectorEngine.memset = lambda self, ap, constant: None
    bass.Bass.all_engine_barrier = lambda self, **kw: None
    try:
        nc = bass.Bass(enable_partition_id=False)
    finally:
        bass.BassEitherVectorEngine.memset = _orig_memset
        bass.Bass.all_engine_barrier = _orig_aeb
    x = nc.declare_dram_parameter("x", [ROWS, M], mybir.dt.float32, isOutput=False)
    y = nc.declare_dram_parameter("y", [ROWS, M], mybir.dt.float32, isOutput=True)
    rows_per_chunk = ROWS // NCHUNKS
    with nc.Block(no_gpsimd_drain=True) as block, nc.semaphore("dma_sem") as dma_sem:

        @block.sync
        def _(sync: bass.BassEngine):
            for i in range(NCHUNKS):
                sl = slice(i * rows_per_chunk, (i + 1) * rows_per_chunk)
                sync.dma_start(out=y[sl], in_=x[sl]).then_inc(dma_sem, 16)
            sync.wait_ge(dma_sem, 16 * NCHUNKS)

    _cache["nc"] = nc
    return nc


def kernel(inputs: np.ndarray, target: np.ndarray) -> np.ndarray:
    # Forward output == inputs; target only affects the (unused) grad weights.
    x = np.ascontiguousarray(np.asarray(inputs, dtype=np.float32))
    assert x.shape == (N, M), x.shape
    nc = _build()
    shards = x.reshape(NCORES, ROWS, M)
    in_maps = [{"x": shards[i]} for i in range(NCORES)]
    res = run_bass_kernel_spmd(nc, in_maps, list(range(NCORES)))
    return np.concatenate([res.results[i]["y"] for i in range(NCORES)], axis=0)

